# revision 20
# baseline (speedup 1.0000x reference)
"""MultiHeadAttention kernel for 8 trn2 NeuronCores (Bass/Tile).

Problem: B=2, S=2048, E=1024, H=16, D=64 (fp32), boolean mask [B,S,S].
  out = softmax(mask((q W_q^T) (k W_k^T)^T / sqrt(D))) (v W_v^T) W_o^T + b_o

Sharding: batch x head-group. Core c (c = 4*g + r) handles batch g and heads
4r..4r+3. Per core:
  - QKV projections for its 4 heads (fp16 matmuls, fp32 PSUM accumulate)
  - attention in transposed layout (scores.T = [k_tok, q_tok]): PE QK with
    2-head row packing, ACT exp straight out of PSUM, DVE mask multiply
    (fp16, 2x mode), PE AV (2-head column packing) + broadcast-rowsum
    matmuls (all-ones stationary)
  - after each q-block: 4-rank AllGather (within the batch group) reshards
    head-rows -> token-slices; all but the last overlap with compute
  - O-projection for this core's 512-token slice; the AllGather output to
    use is selected with a cc_rank-based dynamic DMA offset
Host side does pure layout marshalling (transpose/slice/broadcast/concat).
"""

import sys

sys.path.insert(0, "/opt/trn_rl_repo")

import numpy as np
import concourse.bass as bass
import concourse.mybir as mybir
from concourse import bass_types
from concourse.tile import TileContext
from concourse import bass_utils

F32 = mybir.dt.float32
F16 = mybir.dt.float16
I32 = mybir.dt.int32
AF = mybir.ActivationFunctionType
ALU = mybir.AluOpType

P = 128
E = 1024
HPC = 4  # heads per core
EC = HPC * 64  # e_out columns per core (256)
GROUPS = [[0, 1, 2, 3], [4, 5, 6, 7]]

# walrus limits sync-wait commands per instruction (fp32-class matmuls: 1).
# Split excess waits onto NoOps inserted just before, same engine.
_wait_counter = [0]


def _fix_bir_waits(raw: bytes) -> bytes:
    import orjson

    m = orjson.loads(raw)
    for fn in m["functions"]:
        for blk in fn["blocks"]:
            out = []
            changed = False
            for inst in blk["instructions"]:
                si = inst.get("sync_info") or {}
                waits = si.get("on_wait") or []
                if len(waits) > 1:
                    for w in waits[:-1]:
                        _wait_counter[0] += 1
                        out.append(
                            {
                                "engine": inst["engine"],
                                "ins": [],
                                "name": f"I-waitfix-{_wait_counter[0]}",
                                "opcode": "NoOp",
                                "outs": [],
                                "sync_info": {"on_update": [], "on_wait": [w]},
                            }
                        )
                    si["on_wait"] = waits[-1:]
                    inst["sync_info"] = si
                    changed = True
                out.append(inst)
            if changed:
                blk["instructions"] = out
    return orjson.dumps(m)


def build(S: int = 2048) -> bass.Bass:
    KC = S // 128  # k-chunks
    QBW = S // 4  # q-block width = tokens per rank
    NQB = 4
    NW = min(512, QBW)  # attention matmul moving chunk
    NS = min(512, S)  # projection moving chunk
    MT = min(P, QBW)  # output-row tile

    TSL = QBW // 4  # token-slice width for chunked O-projection (128)

    nc = bass.Bass()

    xqT = nc.declare_dram_parameter("xqT", [E, S], F16, isOutput=False)
    xkT = nc.declare_dram_parameter("xkT", [E, S], F16, isOutput=False)
    xvT = nc.declare_dram_parameter("xvT", [E, S], F16, isOutput=False)
    maskT = nc.declare_dram_parameter("maskT", [S, S], mybir.dt.uint8, isOutput=False)
    WqT = nc.declare_dram_parameter("WqT", [E, EC], F16, isOutput=False)
    WkT = nc.declare_dram_parameter("WkT", [E, EC], F16, isOutput=False)
    WvT = nc.declare_dram_parameter("WvT", [E, EC], F16, isOutput=False)
    WoT = nc.declare_dram_parameter("WoT", [E, E], F16, isOutput=False)
    bq = nc.declare_dram_parameter("bq", [EC], F32, isOutput=False)
    bk = nc.declare_dram_parameter("bk", [EC], F32, isOutput=False)
    bv_b = nc.declare_dram_parameter("bv_b", [P, EC], F32, isOutput=False)
    bo_b = nc.declare_dram_parameter("bo_b", [P, E], F32, isOutput=False)
    out = nc.declare_dram_parameter("out", [QBW, E], F32, isOutput=True)

    with TileContext(nc) as tc:
        with (
            tc.tile_pool(name="persist", bufs=1) as pp,
            tc.tile_pool(name="dramp", bufs=1, space="DRAM") as dramp,
        ):
            # [qb][token-slice 4][2P rows][TSL]; gathered rank-major so
            # ag_out row = qb*4096 + rank*1024 + slice*256 + e
            ag_in = dramp.tile([NQB, 4, 2 * P, TSL], F16)
            ag_out = dramp.tile([NQB * 4 * 4 * 2 * P, TSL], F16)

            qT_sb = pp.tile([P, 2, S], F16)  # [:, m, :] = q.T rows 128m..128m+127
            kT_sb = pp.tile([P, 2, S], F16)
            # [:, t, pr, h, 0:64] = v rows, [:, t, pr, h, 64:128] = 1.0 so
            # the AV matmul replicates the softmax denominator into PSUM
            # rows 64..127 of the same stream (no separate rowsum matmul)
            v_sb = pp.tile([P, KC, 2, 2, P], F16)
            nc.vector.memset(v_sb[:, :, :, :, 64:P], 1.0)
            bq_sb = pp.tile([P, 2], F32)
            bk_sb = pp.tile([P, 2], F32)
            nc.sync.dma_start(bq_sb[:], bq.rearrange("(m p) -> p m", p=P))
            nc.sync.dma_start(bk_sb[:], bk.rearrange("(m p) -> p m", p=P))
            bv_sb = pp.tile([P, EC], F16)
            nc.gpsimd.dma_start(bv_sb[:], bv_b[:])
            bo_sb = pp.tile([P, E], F32)
            nc.sync.dma_start(bo_sb[:], bo_b[:])

            # ---------------- Phase A: QKV projections ----------------
            with (
                tc.tile_pool(name="wpool", bufs=1) as wp,
                tc.tile_pool(name="xpool", bufs=4) as xp,
                tc.tile_pool(name="psA", bufs=8, space="PSUM") as psA,
            ):
                wq_sb = wp.tile([P, 8, EC], F16)
                wk_sb = wp.tile([P, 8, EC], F16)
                wv_sb = wp.tile([P, 8, EC], F16)
                nc.gpsimd.dma_start(wq_sb[:], WqT.rearrange("(kt p) m -> p kt m", p=P))
                nc.gpsimd.dma_start(wk_sb[:], WkT.rearrange("(kt p) m -> p kt m", p=P))
                nc.gpsimd.dma_start(wv_sb[:], WvT.rearrange("(kt p) m -> p kt m", p=P))

                for which in range(3):
                    xT, w_sb = [(xqT, wq_sb), (xkT, wk_sb), (xvT, wv_sb)][which]
                    nps = (2 * S) // NS if which < 2 else KC // 2
                    pst = [
                        psA.tile([P, 512], F32, name=f"psA_{which}_{i}", tag="psA")
                        for i in range(nps)
                    ]
                    for kt in range(8):
                        x_t = xp.tile([P, S], F16, name=f"x_{which}_{kt}", tag="x")
                        # q/k alternate the two HW-DGE queues (SP / Act);
                        # v rides the gpsimd queue, which is free once the
                        # (now fp16) weight loads finish -- 3 parallel x streams
                        if which == 2:
                            dma_eng = nc.gpsimd
                        else:
                            dma_eng = nc.sync if kt % 2 == 0 else nc.scalar
                        x_dma = dma_eng.dma_start(x_t[:], xT[kt * P : (kt + 1) * P, :])
                        if which == 2 and kt == 7:
                            last_x_dma = x_dma
                        if which < 2:
                            # q.T / k.T: out [256, S]; lhsT = W tile, rhs = x.T
                            for m in range(2):
                                lhsT = w_sb[:, kt, m * P : (m + 1) * P]
                                for n in range(S // NS):
                                    nc.tensor.matmul(
                                        pst[m * (S // NS) + n][:, :NS],
                                        lhsT,
                                        x_t[:, n * NS : (n + 1) * NS],
                                        start=(kt == 0),
                                        stop=(kt == 7),
                                    )
                        else:
                            # v: out [S, 256]; lhsT = x.T tile, rhs = W k-tile.
                            # Two token-chunks share one PSUM bank: the
                            # has_written group opens on the even chunk and
                            # closes on the odd one (2KB zero-region rule).
                            for t in range(KC):
                                nc.tensor.matmul(
                                    pst[t // 2][:, (t % 2) * EC : (t % 2 + 1) * EC],
                                    x_t[:, t * P : (t + 1) * P],
                                    w_sb[:, kt, :],
                                    start=(kt == 0 and t % 2 == 0),
                                    stop=(kt == 7 and t % 2 == 1),
                                )
                    if which == 0:
                        for m in range(2):
                            for n in range(S // NS):
                                # (q + bq) / 8, bias before scale
                                nc.vector.tensor_scalar(
                                    qT_sb[:, m, n * NS : (n + 1) * NS],
                                    pst[m * (S // NS) + n][:, :NS],
                                    bq_sb[:, m : m + 1],
                                    0.125,
                                    ALU.add,
                                    ALU.mult,
                                )
                    elif which == 1:
                        for m in range(2):
                            for n in range(S // NS):
                                nc.vector.tensor_scalar(
                                    kT_sb[:, m, n * NS : (n + 1) * NS],
                                    pst[m * (S // NS) + n][:, :NS],
                                    1.0,
                                    bk_sb[:, m : m + 1],
                                    ALU.mult,
                                    ALU.add,
                                )
                    else:
                        for t in range(KC):
                            nc.vector.tensor_tensor(
                                v_sb[:, t, :, :, 0:64],
                                pst[t // 2][
                                    :, (t % 2) * EC : (t % 2 + 1) * EC
                                ].rearrange("p (pr h d) -> p pr h d", pr=2, h=2),
                                bv_sb[:].rearrange(
                                    "p (pr h d) -> p pr h d", pr=2, h=2
                                ),
                                ALU.add,
                            )

            # ---------------- Phase B: attention + per-qb AllGather ----------
            with (
                tc.tile_pool(name="maskpool", bufs=1) as mp,
                tc.tile_pool(name="ppool", bufs=6) as ppl,
                tc.tile_pool(name="epool", bufs=4) as ep,
                tc.tile_pool(name="sps", bufs=2, space="PSUM") as sps,
                tc.tile_pool(name="avps", bufs=2, space="PSUM") as avps,
                tc.tile_pool(name="cpool", bufs=1) as cp,
                tc.tile_pool(name="atpool", bufs=4) as atp,
                tc.tile_pool(name="opool", bufs=2) as op,
            ):
                from concourse.tile_rust import add_dep_helper

                rank = nc.gpsimd.snap(
                    nc.gpsimd.cc_rank(replica_groups=GROUPS), min_val=0, max_val=3
                )
                woT_sb = cp.tile([P, 8, E], F16)
                wo_dma = nc.gpsimd.dma_start(
                    woT_sb[:], WoT.rearrange("(kt p) n -> p kt n", p=P)
                )
                attnT_tiles = {}

                def phase_c_fetch(j):
                    # issue chunk j's dynamic attnT read right after
                    # AllGather j+1 is emitted (AG j long done -> the Pool
                    # completion wait is instant, no queue stall)
                    attnT = atp.tile([P, 2, 4, TSL], F16, name=f"at_{j}", tag="at")
                    attnT_tiles[j] = attnT
                    for h in range(2):
                        base = ag_out[bass.ds(j * 4096 + rank * 256 + h * P, P), :]
                        manual = bass_types.AP(
                            base.tensor,
                            base.offset,
                            [[TSL, P], [1024 * TSL, 4], [1, TSL]],
                        )
                        nc.gpsimd.dma_start(attnT[:, h, :, :], manual)

                def phase_c_chunk(j):
                    # project token-slice #rank of q-block j -> out rows
                    # j*TSL..(j+1)*TSL
                    attnT = attnT_tiles.pop(j)
                    # both 512-col halves live in one 2-bank tile that shares
                    # the avps ring (same shape/tag as the attention tiles)
                    o_ps = avps.tile([P, 2, 512], F32, name=f"o_{j}", tag="av")
                    for n in range(2):
                        for kt in range(8):
                            nc.tensor.matmul(
                                o_ps[:, n, :],
                                attnT[:, kt % 2, kt // 2, :],
                                woT_sb[:, kt, n * 512 : (n + 1) * 512],
                                start=(kt == 0),
                                stop=(kt == 7),
                                skip_group_check=(n == 1),
                            )
                        out_sb = op.tile(
                            [P, 512], F32, name=f"osb_{j}_{n}", tag="osb"
                        )
                        nc.vector.tensor_tensor(
                            out_sb[:, :],
                            o_ps[:, n, :],
                            bo_sb[:, n * 512 : (n + 1) * 512],
                            ALU.add,
                        )
                        nc.sync.dma_start(
                            out[j * TSL : (j + 1) * TSL, n * 512 : (n + 1) * 512],
                            out_sb[:, :],
                        )

                maskbf = mp.tile([P, KC, S], F16)
                for t in range(KC):
                    mdma = nc.gpsimd.dma_start(
                        maskbf[:, t, :], maskT[t * P : (t + 1) * P, :]
                    )
                    if t == 0:
                        # keep the big mask stream off phase A's DMA window:
                        # it has plenty of room to stream during phase B
                        add_dep_helper(
                            mdma.ins,
                            last_x_dma.ins,
                            reason="defer mask load until x loads finish",
                        )

                for qb in range(NQB):
                    qsl = slice(qb * QBW, (qb + 1) * QBW)
                    av_t = [
                        avps.tile(
                            [P, 2, 512], F32, name=f"av_{qb}_{pair}", tag="av"
                        )
                        for pair in range(2)
                    ]
                    # software-pipelined by one step: PE queue order is
                    # QK(i), QK(i+1), AV(i), ... so PE never head-of-line
                    # blocks on exp(i)/mask(i) finishing
                    pending_av = None

                    def flush_av():
                        nonlocal pending_av
                        if pending_av is None:
                            return
                        kc, pair, p_t = pending_av
                        pending_av = None
                        for h in range(2):
                            hsl = slice(h * QBW, (h + 1) * QBW)
                            # [v | ones] stationary: rows 0..63 AV, rows
                            # 64..127 the replicated denominator -- one
                            # stream instead of AV + rowsum
                            nc.tensor.matmul(
                                av_t[pair][:, h, :],
                                v_sb[:, kc, pair, h, :],
                                p_t[:, hsl],
                                start=(kc == 0),
                                stop=(kc == KC - 1),
                                skip_group_check=(h == 1),
                            )

                    for kc in range(KC):
                        ksl = slice(kc * P, (kc + 1) * P)
                        for pair in range(2):
                            s_t = sps.tile(
                                [P, 2, 512], F32, name=f"s_{qb}_{kc}_{pair}", tag="s"
                            )
                            for h in range(2):
                                prt = slice(h * 64, (h + 1) * 64)
                                for n in range(QBW // NW):
                                    nc.tensor.matmul(
                                        s_t[:, h, n * NW : (n + 1) * NW],
                                        kT_sb[prt, pair, ksl],
                                        qT_sb[
                                            prt,
                                            pair,
                                            qb * QBW + n * NW : qb * QBW + (n + 1) * NW,
                                        ],
                                        start=True,
                                        stop=True,
                                    )
                            flush_av()
                            p_t = ppl.tile([P, 2 * QBW], F16, name="p_t", tag="p")
                            nc.scalar.activation(
                                p_t[:].rearrange("p (h n) -> p h n", h=2),
                                s_t[:, :, :QBW],
                                AF.Exp,
                            )
                            nc.vector.tensor_tensor(
                                p_t[:].rearrange("p (h n) -> p h n", h=2),
                                p_t[:].rearrange("p (h n) -> p h n", h=2),
                                maskbf[:, kc, qsl][:, None, :].to_broadcast(
                                    (P, 2, QBW)
                                ),
                                ALU.mult,
                            )
                            pending_av = (kc, pair, p_t)
                    flush_av()
                    # epilogue for this q-block: divide + stage + AllGather
                    for pair in range(2):
                        # repack both heads' denominator rows onto partitions
                        # 0..127 of one tile so a single 512-col reciprocal
                        # covers the pair (recip cost is per-lane columns)
                        d_pack = ep.tile([P, QBW], F32, name="d_pack", tag="dp")
                        for h in range(2):
                            nc.vector.tensor_copy(
                                d_pack[h * 64 : (h + 1) * 64, :],
                                av_t[pair][64:P, h, :],
                            )
                        rb = ep.tile([P, QBW], F32, name="rb", tag="rb")
                        nc.vector.reciprocal(rb[:], d_pack[:])
                        av_f = ep.tile([P, QBW], F16, name="av_f", tag="av_f")
                        for h in range(2):
                            nc.vector.tensor_mul(
                                av_f[h * 64 : (h + 1) * 64, :],
                                av_t[pair][0:64, h, :],
                                rb[h * 64 : (h + 1) * 64, :],
                            )
                        for sl in range(4):
                            nc.sync.dma_start(
                                ag_in[qb, sl, pair * P : (pair + 1) * P, :],
                                av_f[:, sl * TSL : (sl + 1) * TSL],
                            )
                    nc.gpsimd.collective_compute(
                        "AllGather",
                        ALU.bypass,
                        ins=[ag_in[qb]],
                        outs=[ag_out[qb * 4096 : (qb + 1) * 4096, :]],
                        replica_groups=GROUPS,
                    )
                    if qb >= 1:
                        phase_c_fetch(qb - 1)
                    if qb >= 2:
                        phase_c_chunk(qb - 2)
                phase_c_fetch(3)
                phase_c_chunk(2)
                phase_c_chunk(3)

    fixed = _fix_bir_waits(nc.to_json_bytes())
    nc.to_json_bytes = lambda: fixed
    return nc


_NC_CACHE: dict = {}


def _get_nc(S: int) -> bass.Bass:
    if S not in _NC_CACHE:
        _NC_CACHE[S] = build(S)
    return _NC_CACHE[S]


def kernel(
    query,
    key,
    value,
    mask,
    Wq,
    bq,
    Wk,
    bk,
    Wv,
    bv,
    Wo,
    bo,
    _trace: bool = False,
    _trace_dir: str | None = None,
):
    query = np.asarray(query, np.float32)
    key = np.asarray(key, np.float32)
    value = np.asarray(value, np.float32)
    mask = np.ascontiguousarray(np.asarray(mask, np.int32))
    Wq = np.asarray(Wq, np.float32)
    Wk = np.asarray(Wk, np.float32)
    Wv = np.asarray(Wv, np.float32)
    Wo = np.asarray(Wo, np.float32)
    bq = np.asarray(bq, np.float32)
    bk = np.asarray(bk, np.float32)
    bv = np.asarray(bv, np.float32)
    bo = np.asarray(bo, np.float32)

    B, S, E_ = query.shape
    assert (B, E_) == (2, 1024), (B, E_)
    nc = _get_nc(S)

    # host-side layout marshalling (no arithmetic)
    xT = {}
    for g in range(2):
        xT[("q", g)] = np.ascontiguousarray(query[g].T.astype(np.float16))
        xT[("k", g)] = np.ascontiguousarray(key[g].T.astype(np.float16))
        xT[("v", g)] = np.ascontiguousarray(value[g].T.astype(np.float16))
    maskTt = [np.ascontiguousarray(mask[g].T.astype(np.uint8)) for g in range(2)]
    WoT_h = np.ascontiguousarray(Wo.T.astype(np.float16))
    bo_rep = np.ascontiguousarray(np.broadcast_to(bo, (128, 1024)))

    in_maps = []
    for c in range(8):
        g, r = divmod(c, 4)
        hs = slice(r * EC, (r + 1) * EC)
        in_maps.append(
            {
                "xqT": xT[("q", g)],
                "xkT": xT[("k", g)],
                "xvT": xT[("v", g)],
                "maskT": maskTt[g],
                "WqT": np.ascontiguousarray(Wq[hs, :].T.astype(np.float16)),
                "WkT": np.ascontiguousarray(Wk[hs, :].T.astype(np.float16)),
                "WvT": np.ascontiguousarray(Wv[hs, :].T.astype(np.float16)),
                "WoT": WoT_h,
                "bq": np.ascontiguousarray(bq[hs]),
                "bk": np.ascontiguousarray(bk[hs]),
                "bv_b": np.ascontiguousarray(np.broadcast_to(bv[hs], (128, EC))),
                "bo_b": bo_rep,
            }
        )

    kw = {}
    if _trace:
        kw = dict(trace=True, tmpdir=_trace_dir)
    res = bass_utils.run_bass_kernel_spmd(nc, in_maps, list(range(8)), **kw)

    QBW = S // 4
    TSL = QBW // 4
    out_full = np.empty((B, S, E_), np.float32)
    for c in range(8):
        g, r = divmod(c, 4)
        o = res.results[c]["out"]
        for j in range(4):
            out_full[g, j * QBW + r * TSL : j * QBW + (r + 1) * TSL, :] = o[
                j * TSL : (j + 1) * TSL, :
            ]
    if _trace:
        kernel._last_exec_time_ns = res.exec_time_ns
        kernel._last_trace = res.instructions_and_trace
    return out_full



# revision 21
# speedup vs baseline: 1.0647x; 1.0647x over previous
"""MultiHeadAttention kernel for 8 trn2 NeuronCores (Bass/Tile).

Problem: B=2, S=2048, E=1024, H=16, D=64 (fp32), boolean mask [B,S,S].
  out = softmax(mask((q W_q^T) (k W_k^T)^T / sqrt(D))) (v W_v^T) W_o^T + b_o

Sharding: batch x head-group. Core c (c = 4*g + r) handles batch g and heads
4r..4r+3. Per core:
  - QKV projections for its 4 heads (fp16 matmuls, fp32 PSUM accumulate)
  - attention in transposed layout (scores.T = [k_tok, q_tok]): PE QK with
    2-head row packing, ACT exp straight out of PSUM, DVE mask multiply
    (fp16, 2x mode), PE AV (2-head column packing) + broadcast-rowsum
    matmuls (all-ones stationary)
  - after each q-block: 4-rank AllGather (within the batch group) reshards
    head-rows -> token-slices; all but the last overlap with compute
  - O-projection for this core's 512-token slice; the AllGather output to
    use is selected with a cc_rank-based dynamic DMA offset
Host side does pure layout marshalling (transpose/slice/broadcast/concat).
"""

import sys

sys.path.insert(0, "/opt/trn_rl_repo")

import numpy as np
import concourse.bass as bass
import concourse.mybir as mybir
from concourse import bass_types
from concourse.tile import TileContext
from concourse import bass_utils

F32 = mybir.dt.float32
F16 = mybir.dt.float16
I32 = mybir.dt.int32
AF = mybir.ActivationFunctionType
ALU = mybir.AluOpType

P = 128
E = 1024
HPC = 4  # heads per core
EC = HPC * 64  # e_out columns per core (256)
GROUPS = [[0, 1, 2, 3], [4, 5, 6, 7]]

# walrus limits sync-wait commands per instruction (fp32-class matmuls: 1).
# Split excess waits onto NoOps inserted just before, same engine.
_wait_counter = [0]


def _fix_bir_waits(raw: bytes) -> bytes:
    import orjson

    m = orjson.loads(raw)
    for fn in m["functions"]:
        for blk in fn["blocks"]:
            out = []
            changed = False
            for inst in blk["instructions"]:
                si = inst.get("sync_info") or {}
                waits = si.get("on_wait") or []
                if len(waits) > 1:
                    for w in waits[:-1]:
                        _wait_counter[0] += 1
                        out.append(
                            {
                                "engine": inst["engine"],
                                "ins": [],
                                "name": f"I-waitfix-{_wait_counter[0]}",
                                "opcode": "NoOp",
                                "outs": [],
                                "sync_info": {"on_update": [], "on_wait": [w]},
                            }
                        )
                    si["on_wait"] = waits[-1:]
                    inst["sync_info"] = si
                    changed = True
                out.append(inst)
            if changed:
                blk["instructions"] = out
    return orjson.dumps(m)


def build(S: int = 2048) -> bass.Bass:
    KC = S // 128  # k-chunks
    QBW = S // 4  # q-block width = tokens per rank
    NQB = 4
    NW = min(512, QBW)  # attention matmul moving chunk
    NS = min(512, S)  # projection moving chunk
    MT = min(P, QBW)  # output-row tile

    TSL = QBW // 4  # token-slice width for chunked O-projection (128)

    nc = bass.Bass()

    xqT = nc.declare_dram_parameter("xqT", [E, S], F16, isOutput=False)
    xkT = nc.declare_dram_parameter("xkT", [E, S], F16, isOutput=False)
    xvT = nc.declare_dram_parameter("xvT", [E, S], F16, isOutput=False)
    maskT = nc.declare_dram_parameter("maskT", [S, S], mybir.dt.uint8, isOutput=False)
    WqT = nc.declare_dram_parameter("WqT", [E, EC], F16, isOutput=False)
    WkT = nc.declare_dram_parameter("WkT", [E, EC], F16, isOutput=False)
    WvT = nc.declare_dram_parameter("WvT", [E, EC], F16, isOutput=False)
    WoT = nc.declare_dram_parameter("WoT", [E, E], F16, isOutput=False)
    bq = nc.declare_dram_parameter("bq", [EC], F32, isOutput=False)
    bk = nc.declare_dram_parameter("bk", [EC], F32, isOutput=False)
    bv_b = nc.declare_dram_parameter("bv_b", [P, EC], F32, isOutput=False)
    bo_b = nc.declare_dram_parameter("bo_b", [P, E], F32, isOutput=False)
    out = nc.declare_dram_parameter("out", [QBW, E], F32, isOutput=True)

    with TileContext(nc) as tc:
        with (
            tc.tile_pool(name="persist", bufs=1) as pp,
            tc.tile_pool(name="dramp", bufs=1, space="DRAM") as dramp,
        ):
            # [qb][token-slice 4][2P rows][TSL]; gathered rank-major so
            # ag_out row = qb*4096 + rank*1024 + slice*256 + e
            ag_in = dramp.tile([NQB, 4, 2 * P, TSL], F16)
            ag_out = dramp.tile([NQB * 4 * 4 * 2 * P, TSL], F16)

            qT_sb = pp.tile([P, 2, S], F16)  # [:, m, :] = q.T rows 128m..128m+127
            kT_sb = pp.tile([P, 2, S], F16)
            # [:, t, pr, h, 0:64] = v rows, [:, t, pr, h, 64:128] = 1.0 so
            # the AV matmul replicates the softmax denominator into PSUM
            # rows 64..127 of the same stream (no separate rowsum matmul)
            v_sb = pp.tile([P, KC, 2, 2, P], F16)
            nc.vector.memset(v_sb[:, :, :, :, 64:P], 1.0)
            bq_sb = pp.tile([P, 2], F32)
            bk_sb = pp.tile([P, 2], F32)
            nc.sync.dma_start(bq_sb[:], bq.rearrange("(m p) -> p m", p=P))
            nc.sync.dma_start(bk_sb[:], bk.rearrange("(m p) -> p m", p=P))
            bv_sb = pp.tile([P, EC], F16)
            nc.gpsimd.dma_start(bv_sb[:], bv_b[:])
            bo_sb = pp.tile([P, E], F32)
            nc.sync.dma_start(bo_sb[:], bo_b[:])

            # ---------------- Phase A: QKV projections ----------------
            with (
                tc.tile_pool(name="wpool", bufs=1) as wp,
                tc.tile_pool(name="xpool", bufs=4) as xp,
                tc.tile_pool(name="psA", bufs=8, space="PSUM") as psA,
            ):
                wq_sb = wp.tile([P, 8, EC], F16)
                wk_sb = wp.tile([P, 8, EC], F16)
                wv_sb = wp.tile([P, 8, EC], F16)
                nc.gpsimd.dma_start(wq_sb[:], WqT.rearrange("(kt p) m -> p kt m", p=P))
                nc.gpsimd.dma_start(wk_sb[:], WkT.rearrange("(kt p) m -> p kt m", p=P))
                nc.gpsimd.dma_start(wv_sb[:], WvT.rearrange("(kt p) m -> p kt m", p=P))

                for which in range(3):
                    xT, w_sb = [(xqT, wq_sb), (xkT, wk_sb), (xvT, wv_sb)][which]
                    nps = (2 * S) // NS if which < 2 else KC // 2
                    pst = [
                        psA.tile([P, 512], F32, name=f"psA_{which}_{i}", tag="psA")
                        for i in range(nps)
                    ]
                    for kt in range(8):
                        x_t = xp.tile([P, S], F16, name=f"x_{which}_{kt}", tag="x")
                        # q/k alternate the two HW-DGE queues (SP / Act);
                        # v rides the gpsimd queue, which is free once the
                        # (now fp16) weight loads finish -- 3 parallel x streams
                        if which == 2:
                            dma_eng = nc.gpsimd
                        else:
                            dma_eng = nc.sync if kt % 2 == 0 else nc.scalar
                        x_dma = dma_eng.dma_start(x_t[:], xT[kt * P : (kt + 1) * P, :])
                        if which == 2 and kt == 7:
                            last_x_dma = x_dma
                        if which < 2:
                            # q.T / k.T: out [256, S]; lhsT = W tile, rhs = x.T
                            for m in range(2):
                                lhsT = w_sb[:, kt, m * P : (m + 1) * P]
                                for n in range(S // NS):
                                    nc.tensor.matmul(
                                        pst[m * (S // NS) + n][:, :NS],
                                        lhsT,
                                        x_t[:, n * NS : (n + 1) * NS],
                                        start=(kt == 0),
                                        stop=(kt == 7),
                                    )
                        else:
                            # v: out [S, 256]; lhsT = x.T tile, rhs = W k-tile.
                            # Two token-chunks share one PSUM bank: the
                            # has_written group opens on the even chunk and
                            # closes on the odd one (2KB zero-region rule).
                            for t in range(KC):
                                nc.tensor.matmul(
                                    pst[t // 2][:, (t % 2) * EC : (t % 2 + 1) * EC],
                                    x_t[:, t * P : (t + 1) * P],
                                    w_sb[:, kt, :],
                                    start=(kt == 0 and t % 2 == 0),
                                    stop=(kt == 7 and t % 2 == 1),
                                )
                    if which == 0:
                        for m in range(2):
                            for n in range(S // NS):
                                # (q + bq) / 8, bias before scale
                                nc.vector.tensor_scalar(
                                    qT_sb[:, m, n * NS : (n + 1) * NS],
                                    pst[m * (S // NS) + n][:, :NS],
                                    bq_sb[:, m : m + 1],
                                    0.125,
                                    ALU.add,
                                    ALU.mult,
                                )
                    elif which == 1:
                        for m in range(2):
                            for n in range(S // NS):
                                nc.vector.tensor_scalar(
                                    kT_sb[:, m, n * NS : (n + 1) * NS],
                                    pst[m * (S // NS) + n][:, :NS],
                                    1.0,
                                    bk_sb[:, m : m + 1],
                                    ALU.mult,
                                    ALU.add,
                                )
                    else:
                        for t in range(KC):
                            nc.vector.tensor_tensor(
                                v_sb[:, t, :, :, 0:64],
                                pst[t // 2][
                                    :, (t % 2) * EC : (t % 2 + 1) * EC
                                ].rearrange("p (pr h d) -> p pr h d", pr=2, h=2),
                                bv_sb[:].rearrange(
                                    "p (pr h d) -> p pr h d", pr=2, h=2
                                ),
                                ALU.add,
                            )

            # ---------------- Phase B: attention + per-qb AllGather ----------
            with (
                tc.tile_pool(name="maskpool", bufs=1) as mp,
                tc.tile_pool(name="ppool", bufs=6) as ppl,
                tc.tile_pool(name="epool", bufs=4) as ep,
                tc.tile_pool(name="sps", bufs=2, space="PSUM") as sps,
                tc.tile_pool(name="avps", bufs=2, space="PSUM") as avps,
                tc.tile_pool(name="cpool", bufs=1) as cp,
                tc.tile_pool(name="atpool", bufs=4) as atp,
                tc.tile_pool(name="opool", bufs=2) as op,
            ):
                from concourse.tile_rust import add_dep_helper

                rank = nc.gpsimd.snap(
                    nc.gpsimd.cc_rank(replica_groups=GROUPS), min_val=0, max_val=3
                )
                woT_sb = cp.tile([P, 8, E], F16)
                wo_dma = nc.gpsimd.dma_start(
                    woT_sb[:], WoT.rearrange("(kt p) n -> p kt n", p=P)
                )
                attnT_tiles = {}

                def phase_c_fetch(j):
                    # issue chunk j's dynamic attnT read right after
                    # AllGather j+1 is emitted (AG j long done -> the Pool
                    # completion wait is instant, no queue stall)
                    attnT = atp.tile([P, 2, 4, TSL], F16, name=f"at_{j}", tag="at")
                    attnT_tiles[j] = attnT
                    for h in range(2):
                        base = ag_out[bass.ds(j * 4096 + rank * 256 + h * P, P), :]
                        manual = bass_types.AP(
                            base.tensor,
                            base.offset,
                            [[TSL, P], [1024 * TSL, 4], [1, TSL]],
                        )
                        nc.gpsimd.dma_start(attnT[:, h, :, :], manual)

                def phase_c_chunk(j):
                    # project token-slice #rank of q-block j -> out rows
                    # j*TSL..(j+1)*TSL
                    attnT = attnT_tiles.pop(j)
                    # both 512-col halves live in one 2-bank tile that shares
                    # the avps ring (same shape/tag as the attention tiles)
                    o_ps = avps.tile([P, 2, 512], F32, name=f"o_{j}", tag="av")
                    for n in range(2):
                        for kt in range(8):
                            nc.tensor.matmul(
                                o_ps[:, n, :],
                                attnT[:, kt % 2, kt // 2, :],
                                woT_sb[:, kt, n * 512 : (n + 1) * 512],
                                start=(kt == 0),
                                stop=(kt == 7),
                                skip_group_check=(n == 1),
                            )
                        out_sb = op.tile(
                            [P, 512], F32, name=f"osb_{j}_{n}", tag="osb"
                        )
                        nc.vector.tensor_tensor(
                            out_sb[:, :],
                            o_ps[:, n, :],
                            bo_sb[:, n * 512 : (n + 1) * 512],
                            ALU.add,
                        )
                        (nc.sync if n == 0 else nc.scalar).dma_start(
                            out[j * TSL : (j + 1) * TSL, n * 512 : (n + 1) * 512],
                            out_sb[:, :],
                        )

                maskbf = mp.tile([P, KC, S], F16)
                for t in range(KC):
                    mdma = nc.gpsimd.dma_start(
                        maskbf[:, t, :], maskT[t * P : (t + 1) * P, :]
                    )
                    if t == 0:
                        # keep the big mask stream off phase A's DMA window:
                        # it has plenty of room to stream during phase B
                        add_dep_helper(
                            mdma.ins,
                            last_x_dma.ins,
                            reason="defer mask load until x loads finish",
                        )

                for qb in range(NQB):
                    qsl = slice(qb * QBW, (qb + 1) * QBW)
                    av_t = [
                        avps.tile(
                            [P, 2, 512], F32, name=f"av_{qb}_{pair}", tag="av"
                        )
                        for pair in range(2)
                    ]
                    # software-pipelined by one step: PE queue order is
                    # QK(i), QK(i+1), AV(i), ... so PE never head-of-line
                    # blocks on exp(i)/mask(i) finishing
                    pending_av = None

                    def flush_av():
                        nonlocal pending_av
                        if pending_av is None:
                            return
                        kc, pair, p_t = pending_av
                        pending_av = None
                        for h in range(2):
                            hsl = slice(h * QBW, (h + 1) * QBW)
                            # [v | ones] stationary: rows 0..63 AV, rows
                            # 64..127 the replicated denominator -- one
                            # stream instead of AV + rowsum
                            nc.tensor.matmul(
                                av_t[pair][:, h, :],
                                v_sb[:, kc, pair, h, :],
                                p_t[:, hsl],
                                start=(kc == 0),
                                stop=(kc == KC - 1),
                                skip_group_check=(h == 1),
                            )

                    for kc in range(KC):
                        ksl = slice(kc * P, (kc + 1) * P)
                        for pair in range(2):
                            s_t = sps.tile(
                                [P, 2, 512], F32, name=f"s_{qb}_{kc}_{pair}", tag="s"
                            )
                            for h in range(2):
                                prt = slice(h * 64, (h + 1) * 64)
                                for n in range(QBW // NW):
                                    nc.tensor.matmul(
                                        s_t[:, h, n * NW : (n + 1) * NW],
                                        kT_sb[prt, pair, ksl],
                                        qT_sb[
                                            prt,
                                            pair,
                                            qb * QBW + n * NW : qb * QBW + (n + 1) * NW,
                                        ],
                                        start=True,
                                        stop=True,
                                    )
                            flush_av()
                            p_t = ppl.tile([P, 2 * QBW], F16, name="p_t", tag="p")
                            nc.scalar.activation(
                                p_t[:].rearrange("p (h n) -> p h n", h=2),
                                s_t[:, :, :QBW],
                                AF.Exp,
                            )
                            nc.vector.tensor_tensor(
                                p_t[:].rearrange("p (h n) -> p h n", h=2),
                                p_t[:].rearrange("p (h n) -> p h n", h=2),
                                maskbf[:, kc, qsl][:, None, :].to_broadcast(
                                    (P, 2, QBW)
                                ),
                                ALU.mult,
                            )
                            pending_av = (kc, pair, p_t)
                    flush_av()
                    # epilogue for this q-block: divide + stage + AllGather
                    for pair in range(2):
                        # repack both heads' denominator rows onto partitions
                        # 0..127 of one tile so a single 512-col reciprocal
                        # covers the pair (recip cost is per-lane columns)
                        d_pack = ep.tile([P, QBW], F32, name="d_pack", tag="dp")
                        for h in range(2):
                            nc.scalar.copy(
                                d_pack[h * 64 : (h + 1) * 64, :],
                                av_t[pair][64:P, h, :],
                            )
                        rb = ep.tile([P, QBW], F32, name="rb", tag="rb")
                        nc.vector.reciprocal(rb[:], d_pack[:])
                        av_f = ep.tile([P, QBW], F16, name="av_f", tag="av_f")
                        for h in range(2):
                            nc.vector.tensor_mul(
                                av_f[h * 64 : (h + 1) * 64, :],
                                av_t[pair][0:64, h, :],
                                rb[h * 64 : (h + 1) * 64, :],
                            )
                        for sl in range(4):
                            nc.sync.dma_start(
                                ag_in[qb, sl, pair * P : (pair + 1) * P, :],
                                av_f[:, sl * TSL : (sl + 1) * TSL],
                            )
                    nc.gpsimd.collective_compute(
                        "AllGather",
                        ALU.bypass,
                        ins=[ag_in[qb]],
                        outs=[ag_out[qb * 4096 : (qb + 1) * 4096, :]],
                        replica_groups=GROUPS,
                    )
                    if qb >= 1:
                        phase_c_fetch(qb - 1)
                    if qb >= 2:
                        phase_c_chunk(qb - 2)
                phase_c_fetch(3)
                phase_c_chunk(2)
                phase_c_chunk(3)

    fixed = _fix_bir_waits(nc.to_json_bytes())
    nc.to_json_bytes = lambda: fixed
    return nc


_NC_CACHE: dict = {}


def _get_nc(S: int) -> bass.Bass:
    if S not in _NC_CACHE:
        _NC_CACHE[S] = build(S)
    return _NC_CACHE[S]


def kernel(
    query,
    key,
    value,
    mask,
    Wq,
    bq,
    Wk,
    bk,
    Wv,
    bv,
    Wo,
    bo,
    _trace: bool = False,
    _trace_dir: str | None = None,
):
    query = np.asarray(query, np.float32)
    key = np.asarray(key, np.float32)
    value = np.asarray(value, np.float32)
    mask = np.ascontiguousarray(np.asarray(mask, np.int32))
    Wq = np.asarray(Wq, np.float32)
    Wk = np.asarray(Wk, np.float32)
    Wv = np.asarray(Wv, np.float32)
    Wo = np.asarray(Wo, np.float32)
    bq = np.asarray(bq, np.float32)
    bk = np.asarray(bk, np.float32)
    bv = np.asarray(bv, np.float32)
    bo = np.asarray(bo, np.float32)

    B, S, E_ = query.shape
    assert (B, E_) == (2, 1024), (B, E_)
    nc = _get_nc(S)

    # host-side layout marshalling (no arithmetic)
    xT = {}
    for g in range(2):
        xT[("q", g)] = np.ascontiguousarray(query[g].T.astype(np.float16))
        xT[("k", g)] = np.ascontiguousarray(key[g].T.astype(np.float16))
        xT[("v", g)] = np.ascontiguousarray(value[g].T.astype(np.float16))
    maskTt = [np.ascontiguousarray(mask[g].T.astype(np.uint8)) for g in range(2)]
    WoT_h = np.ascontiguousarray(Wo.T.astype(np.float16))
    bo_rep = np.ascontiguousarray(np.broadcast_to(bo, (128, 1024)))

    in_maps = []
    for c in range(8):
        g, r = divmod(c, 4)
        hs = slice(r * EC, (r + 1) * EC)
        in_maps.append(
            {
                "xqT": xT[("q", g)],
                "xkT": xT[("k", g)],
                "xvT": xT[("v", g)],
                "maskT": maskTt[g],
                "WqT": np.ascontiguousarray(Wq[hs, :].T.astype(np.float16)),
                "WkT": np.ascontiguousarray(Wk[hs, :].T.astype(np.float16)),
                "WvT": np.ascontiguousarray(Wv[hs, :].T.astype(np.float16)),
                "WoT": WoT_h,
                "bq": np.ascontiguousarray(bq[hs]),
                "bk": np.ascontiguousarray(bk[hs]),
                "bv_b": np.ascontiguousarray(np.broadcast_to(bv[hs], (128, EC))),
                "bo_b": bo_rep,
            }
        )

    kw = {}
    if _trace:
        kw = dict(trace=True, tmpdir=_trace_dir)
    res = bass_utils.run_bass_kernel_spmd(nc, in_maps, list(range(8)), **kw)

    QBW = S // 4
    TSL = QBW // 4
    out_full = np.empty((B, S, E_), np.float32)
    for c in range(8):
        g, r = divmod(c, 4)
        o = res.results[c]["out"]
        for j in range(4):
            out_full[g, j * QBW + r * TSL : j * QBW + (r + 1) * TSL, :] = o[
                j * TSL : (j + 1) * TSL, :
            ]
    if _trace:
        kernel._last_exec_time_ns = res.exec_time_ns
        kernel._last_trace = res.instructions_and_trace
    return out_full

